# revision 9
# baseline (speedup 1.0000x reference)
"""Attention-pooling kernel (AttLayer) for Trainium2, data-parallel over batch
across 8 NeuronCores.

  uit = tanh(x @ W + b)            [B, T, A]
  ait = exp(uit @ u) * mask        [B, T]
  out = einsum('btd,bt->bd', x, ait / (sum_t ait + eps))

Shapes hardcoded: x [64, 4096, 256] f32, W [256, 32], b [32], u [32, 1],
mask [64, 4096] bool. Each core handles 8 batches.

Layout: per batch, T=4096 rows arrive in 2 contiguous 2MB DMAs of
[128, 16, 256] (partition p holds rows 16p..16p+15 of its half), i.e.
t = 2048 g + 16 p + r.  A "chunk" i = 16 g + r is a [128 t x 256 d] slab
whose within-chunk position is the partition index p.

Per group: one DVE copy converts the whole [128, 16*256] slab f32->bf16.
Per chunk: PE transposes the two [128, 128] d-blocks (bf16 matmul by
identity) into PSUM; DVE/ACT copy them back to SBUF (alternating [128, 512]
tiles to amortize fixed costs); two bf16 PE matmuls accumulate x@W into a
per-batch PSUM region [128, 32*32].  Per batch: DVE adds bias, ACT tanh,
DVE mul by u + reduce -> scores [128, 32]; DVE adds the additive mask bias;
ACT exp -> e (bf16) with fused row-sum accum; 32 bf16 PE matmuls
(e_i^T @ x_i, N=256) accumulate the numerator [1, 256]; one f32 matmul
forms the denominator; reciprocal + scale; DMA out.
"""

import os
import sys

sys.path.insert(0, "/opt/trn_rl_repo")

import numpy as np

import concourse.bass as bass
import concourse.mybir as mybir
import concourse.tile as tile
from concourse import bacc
from concourse.bass import ds, ts
from concourse import bass_utils
from concourse.bass_utils import run_bass_kernel_spmd

if bool(int(os.environ.get("BASS_LDW_OPT", "0"))):
    _orig_run_command = bass_utils.run_command

    def _run_command_ldwopt(argv, **kwargs):
        argv = ["--enable-ldw-opt=true" if a == "--enable-ldw-opt=false" else a
                for a in argv]
        return _orig_run_command(argv, **kwargs)

    bass_utils.run_command = _run_command_ldwopt

F32 = mybir.dt.float32
BF16 = mybir.dt.bfloat16

N_CORES = 8
B, T, D, A = 64, 4096, 256, 32
BPC = B // N_CORES          # batches per core
NCH = T // 128              # 128-row chunks per batch (32)
NG = 2                      # DMA groups per batch
RPG = NCH // NG             # chunks per group (16)
EPS = 1e-7
MASK_BIAS = 30.0            # additive pre-exp mask: s + (mask-1)*30

last_exec_time_ns = None


def _build():
    nc = bacc.Bacc(None, target_bir_lowering=False, debug=True)

    x_dram = nc.dram_tensor("x", [BPC, T, D], F32, kind="ExternalInput")
    w_dram = nc.dram_tensor("w", [128, 2 * A], F32, kind="ExternalInput")
    bbt_dram = nc.dram_tensor("bbt", [128, NCH * A], F32, kind="ExternalInput")
    ubt_dram = nc.dram_tensor("ubt", [128, NCH * A], F32, kind="ExternalInput")
    maskb_dram = nc.dram_tensor("maskb", [BPC, 128, NCH], F32, kind="ExternalInput")
    ident_dram = nc.dram_tensor("ident", [128, 128], F32, kind="ExternalInput")
    out_dram = nc.dram_tensor("out", [BPC, D], F32, kind="ExternalOutput")

    with tile.TileContext(nc) as tc:
        with (
            tc.tile_pool(name="const", bufs=1) as cpool,
            tc.tile_pool(name="xf", bufs=3) as xfpool,
            tc.tile_pool(name="xb", bufs=4) as xbpool,
            tc.tile_pool(name="xt", bufs=4) as xtpool,
            tc.tile_pool(name="ph2", bufs=2) as ph2pool,
            tc.tile_pool(name="small", bufs=2) as spool,
            tc.tile_pool(name="uitps", bufs=2, space="PSUM") as uitpool,
            tc.tile_pool(name="xtps", bufs=2, space="PSUM") as xtpspool,
            tc.tile_pool(name="ops", bufs=1, space="PSUM") as opool,
            tc.tile_pool(name="denps", bufs=1, space="PSUM") as denpool,
        ):
            # ---- constants (one-time) ----
            w_f32 = cpool.tile([128, 2 * A], F32, name="w_f32")
            nc.sync.dma_start(out=w_f32[:], in_=w_dram[:])
            w_bf = cpool.tile([128, 2 * A], BF16, name="w_bf")
            nc.vector.tensor_copy(w_bf[:], w_f32[:])

            ident = cpool.tile([128, 128], F32, name="ident")
            nc.sync.dma_start(out=ident[:], in_=ident_dram[:])
            ident_bf = cpool.tile([128, 128], BF16, name="ident_bf")
            nc.vector.tensor_copy(ident_bf[:], ident[:])

            bbt = cpool.tile([128, NCH * A], F32, name="bbt")
            nc.sync.dma_start(out=bbt[:], in_=bbt_dram[:])
            ubt = cpool.tile([128, NCH * A], F32, name="ubt")
            nc.sync.dma_start(out=ubt[:], in_=ubt_dram[:])

            ones_f = cpool.tile([128, 1], F32, name="ones_f")
            nc.vector.memset(ones_f[:], 1.0)

            for bb in range(BPC):
                uit_ps = uitpool.tile([128, NCH * A], F32, name="uit_ps", tag="uit")
                x_bf_tiles = []
                for g in range(NG):
                    x_grp = xfpool.tile([128, RPG, D], F32, name="x_grp", tag="xf")
                    nc.sync.dma_start(
                        out=x_grp[:],
                        in_=x_dram[bb][ds(2048 * g, 2048), :].rearrange(
                            "(p r) d -> p r d", r=RPG
                        ),
                    )
                    x_bf = xbpool.tile([128, RPG, D], BF16, name="x_bf", tag="xb")
                    nc.vector.tensor_copy(x_bf[:], x_grp[:])
                    x_bf_tiles.append(x_bf)

                    # transpose chunks in pairs via regular matmul-by-identity
                    # (normal-mode MMs keep the PE HAM-warm; transpose-mode
                    # does not count as PE activity); copy PSUM->SBUF one
                    # [128,512] f32 tile at a time, alternating DVE/ACT
                    for rp in range(RPG // 2):
                        xt_ps = xtpspool.tile([128, 2, D], F32, name="xt_ps", tag="xtps")
                        for rr in range(2):
                            r = 2 * rp + rr
                            for dc in range(2):
                                nc.tensor.matmul(
                                    xt_ps[:, rr, ds(128 * dc, 128)],
                                    lhsT=x_bf[:, r, ds(128 * dc, 128)],
                                    rhs=ident_bf[:],
                                    start=True,
                                    stop=True,
                                )
                        xt_sb = xtpool.tile([128, 2, D], BF16, name="xt_sb", tag="xt")
                        if rp % 2 == 0:
                            nc.vector.tensor_copy(xt_sb[:], xt_ps[:])
                        else:
                            nc.scalar.copy(xt_sb[:], xt_ps[:])
                        for rr in range(2):
                            i = 16 * g + 2 * rp + rr
                            nc.tensor.matmul(
                                uit_ps[:, ds(A * i, A)],
                                lhsT=xt_sb[:, rr, 0:128],
                                rhs=w_bf[:, 0:A],
                                start=True,
                                stop=False,
                            )
                            nc.tensor.matmul(
                                uit_ps[:, ds(A * i, A)],
                                lhsT=xt_sb[:, rr, 128:256],
                                rhs=w_bf[:, A : 2 * A],
                                start=False,
                                stop=True,
                            )

                # ---- phase 2: scores for the whole batch ----
                t1 = ph2pool.tile([128, NCH * A], F32, name="t1", tag="t1")
                nc.vector.tensor_add(t1[:], uit_ps[:], bbt[:])
                t2 = ph2pool.tile([128, NCH * A], F32, name="t2", tag="t2")
                nc.scalar.activation(t2[:], t1[:], mybir.ActivationFunctionType.Tanh)
                t3 = ph2pool.tile([128, NCH * A], F32, name="t3", tag="t3")
                nc.vector.tensor_mul(t3[:], t2[:], ubt[:])
                s_all = spool.tile([128, NCH, 1], F32, name="s_all", tag="s_all")
                nc.vector.reduce_sum(
                    s_all[:],
                    t3.rearrange("p (i a) -> p i a", a=A),
                    axis=mybir.AxisListType.X,
                )

                maskb = spool.tile([128, NCH], F32, name="maskb", tag="maskb")
                nc.sync.dma_start(out=maskb[:], in_=maskb_dram[bb])
                s_m = spool.tile([128, NCH], F32, name="s_m", tag="s_m")
                nc.vector.tensor_add(s_m[:], s_all[:, :, 0], maskb[:])

                e_bf = spool.tile([128, NCH], BF16, name="e_bf", tag="e_bf")
                er = spool.tile([128, 1], F32, name="er", tag="er")
                nc.scalar.activation(
                    e_bf[:],
                    s_m[:],
                    mybir.ActivationFunctionType.Exp,
                    accum_out=er[:],
                )

                den_ps = denpool.tile([1, 1], F32, name="den_ps", tag="den")
                nc.tensor.matmul(
                    den_ps[:], lhsT=er[:], rhs=ones_f[:], start=True, stop=True
                )

                # ---- phase 3: weighted sum over the sequence ----
                o_ps = opool.tile([1, D], F32, name="o_ps", tag="o")
                for i in range(NCH):
                    g, r = divmod(i, RPG)
                    nc.tensor.matmul(
                        o_ps[:],
                        lhsT=e_bf[:, ds(i, 1)],
                        rhs=x_bf_tiles[g][:, r, :],
                        start=(i == 0),
                        stop=(i == NCH - 1),
                    )

                # ---- phase 4: finalize ----
                den_sb = spool.tile([1, 1], F32, name="den_sb", tag="den_sb")
                nc.vector.tensor_scalar_add(den_sb[:], den_ps[:], EPS)
                inv = spool.tile([1, 1], F32, name="inv", tag="inv")
                nc.vector.reciprocal(inv[:], den_sb[:])
                o_sb = spool.tile([1, D], F32, name="o_sb", tag="o_sb")
                nc.vector.tensor_scalar_mul(o_sb[:], o_ps[:], inv[:])
                nc.sync.dma_start(out=out_dram[bb][None, :], in_=o_sb[:])

    nc.finalize()
    return nc


def kernel(x, mask, W, b, u):
    global last_exec_time_ns
    x = np.ascontiguousarray(np.asarray(x), dtype=np.float32)
    mask_f = np.asarray(mask).astype(np.float32)
    W = np.asarray(W, dtype=np.float32)
    b = np.asarray(b, dtype=np.float32)
    u = np.asarray(u, dtype=np.float32)

    # host-side layout prep (all tiny except x, which is only view-sliced)
    w_packed = np.ascontiguousarray(
        W.reshape(2, 128, A).transpose(1, 0, 2).reshape(128, 2 * A)
    )
    bbt = np.ascontiguousarray(np.tile(b[None, :], (128, NCH)))
    ubt = np.ascontiguousarray(np.tile(u[:, 0][None, :], (128, NCH)))
    # mask -> additive pre-exp bias, laid out [b][p][(g r)] with t = 2048g+16p+r
    maskb = np.ascontiguousarray(
        ((mask_f - 1.0) * MASK_BIAS)
        .reshape(B, NG, 128, RPG)
        .transpose(0, 2, 1, 3)
        .reshape(B, 128, NCH)
    )
    ident = np.eye(128, dtype=np.float32)

    nc = _build()

    in_maps = []
    for c in range(N_CORES):
        in_maps.append(
            {
                "x": x[c * BPC : (c + 1) * BPC],
                "w": w_packed,
                "bbt": bbt,
                "ubt": ubt,
                "maskb": maskb[c * BPC : (c + 1) * BPC],
                "ident": ident,
            }
        )

    trace = bool(int(os.environ.get("BASS_KERNEL_TRACE", "0")))
    res = run_bass_kernel_spmd(
        nc, in_maps, core_ids=list(range(N_CORES)), trace=trace
    )
    last_exec_time_ns = res.exec_time_ns

    out = np.empty((B, D), dtype=np.float32)
    for c in range(N_CORES):
        out[c * BPC : (c + 1) * BPC] = res.results[c]["out"]
    return out


# revision 10
# speedup vs baseline: 1.2233x; 1.2233x over previous
"""Attention-pooling kernel (AttLayer) for Trainium2, data-parallel over batch
across 8 NeuronCores.

  uit = tanh(x @ W + b)            [B, T, A]
  ait = exp(uit @ u) * mask        [B, T]
  out = einsum('btd,bt->bd', x, ait / (sum_t ait + eps))

Shapes hardcoded: x [64, 4096, 256] f32, W [256, 32], b [32], u [32, 1],
mask [64, 4096] bool. Each core handles 8 batches.

Layout: per batch, T=4096 rows arrive in 2 contiguous 2MB DMAs of
[128, 16, 256] (partition p holds rows 16p..16p+15 of its half), i.e.
t = 2048 g + 16 p + r.  A "chunk" i = 16 g + r is a [128 t x 256 d] slab
whose within-chunk position is the partition index p.

Per group: one DVE copy converts the whole [128, 16*256] slab f32->bf16.
Per chunk: PE transposes the two [128, 128] d-blocks (bf16 matmul by
identity) into PSUM; DVE/ACT copy them back to SBUF (alternating [128, 512]
tiles to amortize fixed costs); two bf16 PE matmuls accumulate x@W into a
per-batch PSUM region [128, 32*32].  Per batch: DVE adds bias, ACT tanh,
DVE mul by u + reduce -> scores [128, 32]; DVE adds the additive mask bias;
ACT exp -> e (bf16) with fused row-sum accum; 32 bf16 PE matmuls
(e_i^T @ x_i, N=256) accumulate the numerator [1, 256]; one f32 matmul
forms the denominator; reciprocal + scale; DMA out.
"""

import os
import sys

sys.path.insert(0, "/opt/trn_rl_repo")

import numpy as np

import concourse.bass as bass
import concourse.mybir as mybir
import concourse.tile as tile
from concourse import bacc
from concourse.bass import ds, ts
from concourse import bass_utils
from concourse.bass_utils import run_bass_kernel_spmd

if bool(int(os.environ.get("BASS_LDW_OPT", "0"))):
    _orig_run_command = bass_utils.run_command

    def _run_command_ldwopt(argv, **kwargs):
        argv = ["--enable-ldw-opt=true" if a == "--enable-ldw-opt=false" else a
                for a in argv]
        return _orig_run_command(argv, **kwargs)

    bass_utils.run_command = _run_command_ldwopt

F32 = mybir.dt.float32
BF16 = mybir.dt.bfloat16

N_CORES = 8
B, T, D, A = 64, 4096, 256, 32
BPC = B // N_CORES          # batches per core
NCH = T // 128              # 128-row chunks per batch (32)
NG = 2                      # DMA groups per batch
RPG = NCH // NG             # chunks per group (16)
EPS = 1e-7
MASK_BIAS = 30.0            # additive pre-exp mask: s + (mask-1)*30

last_exec_time_ns = None


def _build():
    nc = bacc.Bacc(None, target_bir_lowering=False, debug=True)

    x_dram = nc.dram_tensor("x", [BPC, T, D], F32, kind="ExternalInput")
    w_dram = nc.dram_tensor("w", [128, 2 * A], F32, kind="ExternalInput")
    bbt_dram = nc.dram_tensor("bbt", [128, NCH * A], F32, kind="ExternalInput")
    ubt_dram = nc.dram_tensor("ubt", [128, NCH * A], F32, kind="ExternalInput")
    maskb_dram = nc.dram_tensor("maskb", [BPC, 128, NCH], F32, kind="ExternalInput")
    ident_dram = nc.dram_tensor("ident", [128, 128], F32, kind="ExternalInput")
    out_dram = nc.dram_tensor("out", [BPC, D], F32, kind="ExternalOutput")

    with tile.TileContext(nc) as tc:
        with (
            tc.tile_pool(name="const", bufs=1) as cpool,
            tc.tile_pool(name="xf", bufs=3) as xfpool,
            tc.tile_pool(name="xb", bufs=4) as xbpool,
            tc.tile_pool(name="xt", bufs=4) as xtpool,
            tc.tile_pool(name="ph2", bufs=2) as ph2pool,
            tc.tile_pool(name="small", bufs=2) as spool,
            tc.tile_pool(name="uitps", bufs=2, space="PSUM") as uitpool,
            tc.tile_pool(name="xtps", bufs=2, space="PSUM") as xtpspool,
            tc.tile_pool(name="ops", bufs=1, space="PSUM") as opool,
            tc.tile_pool(name="denps", bufs=1, space="PSUM") as denpool,
        ):
            # ---- constants (one-time) ----
            w_f32 = cpool.tile([128, 2 * A], F32, name="w_f32")
            nc.sync.dma_start(out=w_f32[:], in_=w_dram[:])
            w_bf = cpool.tile([128, 2 * A], BF16, name="w_bf")
            nc.vector.tensor_copy(w_bf[:], w_f32[:])

            ident = cpool.tile([128, 128], F32, name="ident")
            nc.sync.dma_start(out=ident[:], in_=ident_dram[:])
            ident_bf = cpool.tile([128, 128], BF16, name="ident_bf")
            nc.vector.tensor_copy(ident_bf[:], ident[:])

            bbt = cpool.tile([128, NCH * A], F32, name="bbt")
            nc.sync.dma_start(out=bbt[:], in_=bbt_dram[:])
            ubt = cpool.tile([128, NCH * A], F32, name="ubt")
            nc.sync.dma_start(out=ubt[:], in_=ubt_dram[:])

            ones_f = cpool.tile([128, 1], F32, name="ones_f")
            nc.vector.memset(ones_f[:], 1.0)

            for bb in range(BPC):
                uit_ps = uitpool.tile([128, NCH * A], F32, name="uit_ps", tag="uit")
                x_bf_tiles = []
                for g in range(NG):
                    x_grp = xfpool.tile([128, RPG, D], F32, name="x_grp", tag="xf")
                    nc.sync.dma_start(
                        out=x_grp[:],
                        in_=x_dram[bb][ds(2048 * g, 2048), :].rearrange(
                            "(p r) d -> p r d", r=RPG
                        ),
                    )
                    x_bf = xbpool.tile([128, RPG, D], BF16, name="x_bf", tag="xb")
                    nc.vector.tensor_copy(x_bf[:], x_grp[:])
                    x_bf_tiles.append(x_bf)

                    # Transpose 4 chunks per PSUM tile, ONE transpose per
                    # chunk: adjacent bf16 pairs are bitcast to f32 elements,
                    # so a [128t x 128pair] f32 transpose relayouts the whole
                    # [128t x 256d] chunk. PSUM tile = 1 bank; one copy per
                    # tile back to SBUF (as bf16, 2x DVE mode), alternating
                    # DVE/ACT.
                    for rp in range(RPG // 4):
                        xt_ps = xtpspool.tile([128, 4, 128], F32, name="xt_ps", tag="xtps")
                        for rr in range(4):
                            r = 4 * rp + rr
                            nc.tensor.transpose(
                                xt_ps[:, rr, :],
                                x_bf[:, r, :].bitcast(F32),
                                ident[:],
                            )
                        xt_sb = xtpool.tile([128, 4, D], BF16, name="xt_sb", tag="xt")
                        if rp % 2 == 0:
                            nc.vector.tensor_copy(xt_sb[:], xt_ps[:].bitcast(BF16))
                        else:
                            nc.scalar.copy(xt_sb[:], xt_ps[:].bitcast(BF16))
                        for rr in range(4):
                            i = 16 * g + 4 * rp + rr
                            xt = xt_sb[:, rr, :].rearrange("p (t s) -> p s t", s=2)
                            nc.tensor.matmul(
                                uit_ps[:, ds(A * i, A)],
                                lhsT=xt[:, 0, :],
                                rhs=w_bf[:, 0:A],
                                start=True,
                                stop=False,
                            )
                            nc.tensor.matmul(
                                uit_ps[:, ds(A * i, A)],
                                lhsT=xt[:, 1, :],
                                rhs=w_bf[:, A : 2 * A],
                                start=False,
                                stop=True,
                            )

                # ---- phase 2: scores for the whole batch ----
                t1 = ph2pool.tile([128, NCH * A], F32, name="t1", tag="t1")
                nc.vector.tensor_add(t1[:], uit_ps[:], bbt[:])
                t2 = ph2pool.tile([128, NCH * A], F32, name="t2", tag="t2")
                nc.scalar.activation(t2[:], t1[:], mybir.ActivationFunctionType.Tanh)
                t3 = ph2pool.tile([128, NCH * A], F32, name="t3", tag="t3")
                nc.vector.tensor_mul(t3[:], t2[:], ubt[:])
                s_all = spool.tile([128, NCH, 1], F32, name="s_all", tag="s_all")
                nc.vector.reduce_sum(
                    s_all[:],
                    t3.rearrange("p (i a) -> p i a", a=A),
                    axis=mybir.AxisListType.X,
                )

                maskb = spool.tile([128, NCH], F32, name="maskb", tag="maskb")
                nc.sync.dma_start(out=maskb[:], in_=maskb_dram[bb])
                s_m = spool.tile([128, NCH], F32, name="s_m", tag="s_m")
                nc.vector.tensor_add(s_m[:], s_all[:, :, 0], maskb[:])

                e_bf = spool.tile([128, NCH], BF16, name="e_bf", tag="e_bf")
                er = spool.tile([128, 1], F32, name="er", tag="er")
                nc.scalar.activation(
                    e_bf[:],
                    s_m[:],
                    mybir.ActivationFunctionType.Exp,
                    accum_out=er[:],
                )

                den_ps = denpool.tile([1, 1], F32, name="den_ps", tag="den")
                nc.tensor.matmul(
                    den_ps[:], lhsT=er[:], rhs=ones_f[:], start=True, stop=True
                )

                # ---- phase 3: weighted sum over the sequence ----
                o_ps = opool.tile([1, D], F32, name="o_ps", tag="o")
                for i in range(NCH):
                    g, r = divmod(i, RPG)
                    nc.tensor.matmul(
                        o_ps[:],
                        lhsT=e_bf[:, ds(i, 1)],
                        rhs=x_bf_tiles[g][:, r, :],
                        start=(i == 0),
                        stop=(i == NCH - 1),
                    )

                # ---- phase 4: finalize ----
                den_sb = spool.tile([1, 1], F32, name="den_sb", tag="den_sb")
                nc.vector.tensor_scalar_add(den_sb[:], den_ps[:], EPS)
                inv = spool.tile([1, 1], F32, name="inv", tag="inv")
                nc.vector.reciprocal(inv[:], den_sb[:])
                o_sb = spool.tile([1, D], F32, name="o_sb", tag="o_sb")
                nc.vector.tensor_scalar_mul(o_sb[:], o_ps[:], inv[:])
                nc.sync.dma_start(out=out_dram[bb][None, :], in_=o_sb[:])

    nc.finalize()
    return nc


def kernel(x, mask, W, b, u):
    global last_exec_time_ns
    x = np.ascontiguousarray(np.asarray(x), dtype=np.float32)
    mask_f = np.asarray(mask).astype(np.float32)
    W = np.asarray(W, dtype=np.float32)
    b = np.asarray(b, dtype=np.float32)
    u = np.asarray(u, dtype=np.float32)

    # host-side layout prep (all tiny except x, which is only view-sliced)
    # pair packing: w_packed[j, s*A + a] = W[2j + s, a]
    w_packed = np.ascontiguousarray(W.reshape(128, 2 * A))
    bbt = np.ascontiguousarray(np.tile(b[None, :], (128, NCH)))
    ubt = np.ascontiguousarray(np.tile(u[:, 0][None, :], (128, NCH)))
    # mask -> additive pre-exp bias, laid out [b][p][(g r)] with t = 2048g+16p+r
    maskb = np.ascontiguousarray(
        ((mask_f - 1.0) * MASK_BIAS)
        .reshape(B, NG, 128, RPG)
        .transpose(0, 2, 1, 3)
        .reshape(B, 128, NCH)
    )
    ident = np.eye(128, dtype=np.float32)

    nc = _build()

    in_maps = []
    for c in range(N_CORES):
        in_maps.append(
            {
                "x": x[c * BPC : (c + 1) * BPC],
                "w": w_packed,
                "bbt": bbt,
                "ubt": ubt,
                "maskb": maskb[c * BPC : (c + 1) * BPC],
                "ident": ident,
            }
        )

    trace = bool(int(os.environ.get("BASS_KERNEL_TRACE", "0")))
    res = run_bass_kernel_spmd(
        nc, in_maps, core_ids=list(range(N_CORES)), trace=trace
    )
    last_exec_time_ns = res.exec_time_ns

    out = np.empty((B, D), dtype=np.float32)
    for c in range(N_CORES):
        out[c * BPC : (c + 1) * BPC] = res.results[c]["out"]
    return out


# revision 12
# speedup vs baseline: 1.2291x; 1.0047x over previous
"""Attention-pooling kernel (AttLayer) for Trainium2, data-parallel over batch
across 8 NeuronCores.

  uit = tanh(x @ W + b)            [B, T, A]
  ait = exp(uit @ u) * mask        [B, T]
  out = einsum('btd,bt->bd', x, ait / (sum_t ait + eps))

Shapes hardcoded: x [64, 4096, 256] f32, W [256, 32], b [32], u [32, 1],
mask [64, 4096] bool. Each core handles 8 batches.

Layout: per batch, T=4096 rows arrive in 2 contiguous 2MB DMAs of
[128, 16, 256] (partition p holds rows 16p..16p+15 of its half), i.e.
t = 2048 g + 16 p + r.  A "chunk" i = 16 g + r is a [128 t x 256 d] slab
whose within-chunk position is the partition index p.

Per group: one DVE copy converts the whole [128, 16*256] slab f32->bf16.
Per chunk: PE transposes the two [128, 128] d-blocks (bf16 matmul by
identity) into PSUM; DVE/ACT copy them back to SBUF (alternating [128, 512]
tiles to amortize fixed costs); two bf16 PE matmuls accumulate x@W into a
per-batch PSUM region [128, 32*32].  Per batch: DVE adds bias, ACT tanh,
DVE mul by u + reduce -> scores [128, 32]; DVE adds the additive mask bias;
ACT exp -> e (bf16) with fused row-sum accum; 32 bf16 PE matmuls
(e_i^T @ x_i, N=256) accumulate the numerator [1, 256]; one f32 matmul
forms the denominator; reciprocal + scale; DMA out.
"""

import os
import sys

sys.path.insert(0, "/opt/trn_rl_repo")

import numpy as np

import concourse.bass as bass
import concourse.mybir as mybir
import concourse.tile as tile
from concourse import bacc
from concourse.bass import ds, ts
from concourse import bass_utils
from concourse.bass_utils import run_bass_kernel_spmd

if bool(int(os.environ.get("BASS_LDW_OPT", "0"))):
    _orig_run_command = bass_utils.run_command

    def _run_command_ldwopt(argv, **kwargs):
        argv = ["--enable-ldw-opt=true" if a == "--enable-ldw-opt=false" else a
                for a in argv]
        return _orig_run_command(argv, **kwargs)

    bass_utils.run_command = _run_command_ldwopt

F32 = mybir.dt.float32
BF16 = mybir.dt.bfloat16

N_CORES = 8
B, T, D, A = 64, 4096, 256, 32
BPC = B // N_CORES          # batches per core
NCH = T // 128              # 128-row chunks per batch (32)
NG = 2                      # DMA groups per batch
RPG = NCH // NG             # chunks per group (16)
EPS = 1e-7
MASK_BIAS = 30.0            # additive pre-exp mask: s + (mask-1)*30

last_exec_time_ns = None


def _build():
    nc = bacc.Bacc(None, target_bir_lowering=False, debug=True)

    x_dram = nc.dram_tensor("x", [BPC, T, D], F32, kind="ExternalInput")
    w_dram = nc.dram_tensor("w", [128, 2 * A], F32, kind="ExternalInput")
    bbt_dram = nc.dram_tensor("bbt", [128, NCH * A], F32, kind="ExternalInput")
    ubt_dram = nc.dram_tensor("ubt", [128, NCH * A], F32, kind="ExternalInput")
    maskb_dram = nc.dram_tensor("maskb", [BPC, 128, NCH], F32, kind="ExternalInput")
    ident_dram = nc.dram_tensor("ident", [128, 128], F32, kind="ExternalInput")
    out_dram = nc.dram_tensor("out", [BPC, D], F32, kind="ExternalOutput")

    with tile.TileContext(nc) as tc:
        with (
            tc.tile_pool(name="const", bufs=1) as cpool,
            tc.tile_pool(name="xf", bufs=3) as xfpool,
            tc.tile_pool(name="xb", bufs=4) as xbpool,
            tc.tile_pool(name="xt", bufs=4) as xtpool,
            tc.tile_pool(name="ph2", bufs=2) as ph2pool,
            tc.tile_pool(name="small", bufs=2) as spool,
            tc.tile_pool(name="uitps", bufs=2, space="PSUM") as uitpool,
            tc.tile_pool(name="xtps", bufs=4, space="PSUM") as xtpspool,
            tc.tile_pool(name="ops", bufs=1, space="PSUM") as opool,
            tc.tile_pool(name="denps", bufs=1, space="PSUM") as denpool,
        ):
            # ---- constants (one-time) ----
            w_f32 = cpool.tile([128, 2 * A], F32, name="w_f32")
            nc.sync.dma_start(out=w_f32[:], in_=w_dram[:])
            w_bf = cpool.tile([128, 2 * A], BF16, name="w_bf")
            nc.vector.tensor_copy(w_bf[:], w_f32[:])

            ident = cpool.tile([128, 128], F32, name="ident")
            nc.sync.dma_start(out=ident[:], in_=ident_dram[:])
            ident_bf = cpool.tile([128, 128], BF16, name="ident_bf")
            nc.vector.tensor_copy(ident_bf[:], ident[:])

            bbt = cpool.tile([128, NCH * A], F32, name="bbt")
            nc.sync.dma_start(out=bbt[:], in_=bbt_dram[:])
            ubt = cpool.tile([128, NCH * A], F32, name="ubt")
            nc.sync.dma_start(out=ubt[:], in_=ubt_dram[:])

            ones_f = cpool.tile([128, 1], F32, name="ones_f")
            nc.vector.memset(ones_f[:], 1.0)

            for bb in range(BPC):
                uit_ps_halves = []
                x_bf_tiles = []
                for g in range(NG):
                    uit_ps = uitpool.tile([128, RPG * A], F32, name="uit_ps", tag="uit")
                    uit_ps_halves.append(uit_ps)
                    x_grp = xfpool.tile([128, RPG, D], F32, name="x_grp", tag="xf")
                    nc.sync.dma_start(
                        out=x_grp[:],
                        in_=x_dram[bb][ds(2048 * g, 2048), :].rearrange(
                            "(p r) d -> p r d", r=RPG
                        ),
                    )
                    x_bf = xbpool.tile([128, RPG, D], BF16, name="x_bf", tag="xb")
                    nc.vector.tensor_copy(x_bf[:], x_grp[:])
                    x_bf_tiles.append(x_bf)

                    # Transpose 4 chunks per PSUM tile, ONE transpose per
                    # chunk: adjacent bf16 pairs are bitcast to f32 elements,
                    # so a [128t x 128pair] f32 transpose relayouts the whole
                    # [128t x 256d] chunk. PSUM tile = 1 bank; one copy per
                    # tile back to SBUF (as bf16, 2x DVE mode), alternating
                    # DVE/ACT.
                    for rp in range(RPG // 4):
                        xt_ps = xtpspool.tile([128, 4, 128], F32, name="xt_ps", tag="xtps")
                        for rr in range(4):
                            r = 4 * rp + rr
                            nc.tensor.transpose(
                                xt_ps[:, rr, :],
                                x_bf[:, r, :].bitcast(F32),
                                ident[:],
                            )
                        xt_sb = xtpool.tile([128, 4, D], BF16, name="xt_sb", tag="xt")
                        if rp % 2 == 0:
                            nc.vector.tensor_copy(xt_sb[:], xt_ps[:].bitcast(BF16))
                        else:
                            nc.scalar.copy(xt_sb[:], xt_ps[:].bitcast(BF16))
                        for rr in range(4):
                            i = 4 * rp + rr
                            xt = xt_sb[:, rr, :].rearrange("p (t s) -> p s t", s=2)
                            nc.tensor.matmul(
                                uit_ps[:, ds(A * i, A)],
                                lhsT=xt[:, 0, :],
                                rhs=w_bf[:, 0:A],
                                start=True,
                                stop=False,
                            )
                            nc.tensor.matmul(
                                uit_ps[:, ds(A * i, A)],
                                lhsT=xt[:, 1, :],
                                rhs=w_bf[:, A : 2 * A],
                                start=False,
                                stop=True,
                            )

                # ---- phase 2: scores, one half-batch at a time ----
                s_all = spool.tile([128, NCH, 1], F32, name="s_all", tag="s_all")
                for g in range(NG):
                    t1 = ph2pool.tile([128, RPG * A], F32, name="t1", tag="t1")
                    nc.vector.tensor_add(
                        t1[:], uit_ps_halves[g][:], bbt[:, ds(RPG * A * g, RPG * A)]
                    )
                    t2 = ph2pool.tile([128, RPG * A], F32, name="t2", tag="t2")
                    nc.scalar.activation(
                        t2[:], t1[:], mybir.ActivationFunctionType.Tanh
                    )
                    t3 = ph2pool.tile([128, RPG * A], F32, name="t3", tag="t3")
                    nc.vector.tensor_mul(t3[:], t2[:], ubt[:, ds(RPG * A * g, RPG * A)])
                    nc.vector.reduce_sum(
                        s_all[:, ds(RPG * g, RPG), :],
                        t3.rearrange("p (i a) -> p i a", a=A),
                        axis=mybir.AxisListType.X,
                    )

                maskb = spool.tile([128, NCH], F32, name="maskb", tag="maskb")
                nc.sync.dma_start(out=maskb[:], in_=maskb_dram[bb])
                s_m = spool.tile([128, NCH], F32, name="s_m", tag="s_m")
                nc.vector.tensor_add(s_m[:], s_all[:, :, 0], maskb[:])

                e_bf = spool.tile([128, NCH], BF16, name="e_bf", tag="e_bf")
                er = spool.tile([128, 1], F32, name="er", tag="er")
                nc.scalar.activation(
                    e_bf[:],
                    s_m[:],
                    mybir.ActivationFunctionType.Exp,
                    accum_out=er[:],
                )

                den_ps = denpool.tile([1, 1], F32, name="den_ps", tag="den")
                nc.tensor.matmul(
                    den_ps[:], lhsT=er[:], rhs=ones_f[:], start=True, stop=True
                )

                # ---- phase 3: weighted sum, two chunks per matmul ----
                # out[2, 512] = [e_i0 e_i1]^T @ [x_i0 | x_i1]; the diagonal
                # blocks (row 0 left, row 1 right) are the real sums, the
                # off-diagonal blocks are discarded.
                o_ps = opool.tile([2, 2 * D], F32, name="o_ps", tag="o")
                for q in range(NCH // 2):
                    g, r0 = divmod(2 * q, RPG)
                    nc.tensor.matmul(
                        o_ps[:],
                        lhsT=e_bf[:, ds(2 * q, 2)],
                        rhs=x_bf_tiles[g][:, r0 : r0 + 2, :],
                        start=(q == 0),
                        stop=(q == NCH // 2 - 1),
                    )

                # ---- phase 4: finalize ----
                den_sb = spool.tile([1, 1], F32, name="den_sb", tag="den_sb")
                nc.vector.tensor_scalar_add(den_sb[:], den_ps[:], EPS)
                inv = spool.tile([1, 1], F32, name="inv", tag="inv")
                nc.vector.reciprocal(inv[:], den_sb[:])
                o2_sb = spool.tile([2, 2 * D], F32, name="o2_sb", tag="o2_sb")
                nc.vector.tensor_copy(o2_sb[:], o_ps[:])
                o_hi = spool.tile([1, D], F32, name="o_hi", tag="o_hi")
                nc.sync.dma_start(out=o_hi[:], in_=o2_sb[1:2, ds(D, D)])
                o_sum = spool.tile([1, D], F32, name="o_sum", tag="o_sum")
                nc.vector.tensor_add(o_sum[:], o2_sb[0:1, 0:D], o_hi[:])
                o_sb = spool.tile([1, D], F32, name="o_sb", tag="o_sb")
                nc.vector.tensor_scalar_mul(o_sb[:], o_sum[:], inv[:])
                nc.sync.dma_start(out=out_dram[bb][None, :], in_=o_sb[:])

    nc.finalize()
    return nc


def kernel(x, mask, W, b, u):
    global last_exec_time_ns
    x = np.ascontiguousarray(np.asarray(x), dtype=np.float32)
    mask_f = np.asarray(mask).astype(np.float32)
    W = np.asarray(W, dtype=np.float32)
    b = np.asarray(b, dtype=np.float32)
    u = np.asarray(u, dtype=np.float32)

    # host-side layout prep (all tiny except x, which is only view-sliced)
    # pair packing: w_packed[j, s*A + a] = W[2j + s, a]
    w_packed = np.ascontiguousarray(W.reshape(128, 2 * A))
    bbt = np.ascontiguousarray(np.tile(b[None, :], (128, NCH)))
    ubt = np.ascontiguousarray(np.tile(u[:, 0][None, :], (128, NCH)))
    # mask -> additive pre-exp bias, laid out [b][p][(g r)] with t = 2048g+16p+r
    maskb = np.ascontiguousarray(
        ((mask_f - 1.0) * MASK_BIAS)
        .reshape(B, NG, 128, RPG)
        .transpose(0, 2, 1, 3)
        .reshape(B, 128, NCH)
    )
    ident = np.eye(128, dtype=np.float32)

    nc = _build()

    in_maps = []
    for c in range(N_CORES):
        in_maps.append(
            {
                "x": x[c * BPC : (c + 1) * BPC],
                "w": w_packed,
                "bbt": bbt,
                "ubt": ubt,
                "maskb": maskb[c * BPC : (c + 1) * BPC],
                "ident": ident,
            }
        )

    trace = bool(int(os.environ.get("BASS_KERNEL_TRACE", "0")))
    res = run_bass_kernel_spmd(
        nc, in_maps, core_ids=list(range(N_CORES)), trace=trace
    )
    last_exec_time_ns = res.exec_time_ns

    out = np.empty((B, D), dtype=np.float32)
    for c in range(N_CORES):
        out[c * BPC : (c + 1) * BPC] = res.results[c]["out"]
    return out


# revision 13
# speedup vs baseline: 1.3299x; 1.0820x over previous
"""Attention-pooling kernel (AttLayer) for Trainium2, data-parallel over batch
across 8 NeuronCores.

  uit = tanh(x @ W + b)            [B, T, A]
  ait = exp(uit @ u) * mask        [B, T]
  out = einsum('btd,bt->bd', x, ait / (sum_t ait + eps))

Shapes hardcoded: x [64, 4096, 256] f32, W [256, 32], b [32], u [32, 1],
mask [64, 4096] bool. Each core handles 8 batches.

Layout: per batch, T=4096 rows arrive in 2 contiguous 2MB DMAs of
[128, 16, 256] (partition p holds rows 16p..16p+15 of its half), i.e.
t = 2048 g + 16 p + r.  A "chunk" i = 16 g + r is a [128 t x 256 d] slab
whose within-chunk position is the partition index p.

Per group: one DVE copy converts the whole [128, 16*256] slab f32->bf16.
Per chunk: PE transposes the two [128, 128] d-blocks (bf16 matmul by
identity) into PSUM; DVE/ACT copy them back to SBUF (alternating [128, 512]
tiles to amortize fixed costs); two bf16 PE matmuls accumulate x@W into a
per-batch PSUM region [128, 32*32].  Per batch: DVE adds bias, ACT tanh,
DVE mul by u + reduce -> scores [128, 32]; DVE adds the additive mask bias;
ACT exp -> e (bf16) with fused row-sum accum; 32 bf16 PE matmuls
(e_i^T @ x_i, N=256) accumulate the numerator [1, 256]; one f32 matmul
forms the denominator; reciprocal + scale; DMA out.
"""

import os
import sys

sys.path.insert(0, "/opt/trn_rl_repo")

import numpy as np

import concourse.bass as bass
import concourse.mybir as mybir
import concourse.tile as tile
from concourse import bacc
from concourse.bass import ds, ts
from concourse import bass_utils
from concourse.bass_utils import run_bass_kernel_spmd

if bool(int(os.environ.get("BASS_LDW_OPT", "0"))):
    _orig_run_command = bass_utils.run_command

    def _run_command_ldwopt(argv, **kwargs):
        argv = ["--enable-ldw-opt=true" if a == "--enable-ldw-opt=false" else a
                for a in argv]
        return _orig_run_command(argv, **kwargs)

    bass_utils.run_command = _run_command_ldwopt

F32 = mybir.dt.float32
BF16 = mybir.dt.bfloat16

N_CORES = 8
B, T, D, A = 64, 4096, 256, 32
BPC = B // N_CORES          # batches per core
NCH = T // 128              # 128-row chunks per batch (32)
NG = 2                      # DMA groups per batch
RPG = NCH // NG             # chunks per group (16)
EPS = 1e-7
MASK_BIAS = 30.0            # additive pre-exp mask: s + (mask-1)*30

last_exec_time_ns = None


def _build():
    nc = bacc.Bacc(None, target_bir_lowering=False, debug=True)

    x_dram = nc.dram_tensor("x", [BPC, T, D], F32, kind="ExternalInput")
    w_dram = nc.dram_tensor("w", [128, 2 * A], F32, kind="ExternalInput")
    bbt_dram = nc.dram_tensor("bbt", [1, NCH * A], F32, kind="ExternalInput")
    ubt_dram = nc.dram_tensor("ubt", [128, NCH * A], F32, kind="ExternalInput")
    maskb_dram = nc.dram_tensor("maskb", [BPC, 128, NCH], F32, kind="ExternalInput")
    ident_dram = nc.dram_tensor("ident", [128, 128], F32, kind="ExternalInput")
    out_dram = nc.dram_tensor("out", [BPC, D], F32, kind="ExternalOutput")

    with tile.TileContext(nc) as tc:
        with (
            tc.tile_pool(name="const", bufs=1) as cpool,
            tc.tile_pool(name="xf", bufs=3) as xfpool,
            tc.tile_pool(name="xb", bufs=4) as xbpool,
            tc.tile_pool(name="xt", bufs=4) as xtpool,
            tc.tile_pool(name="ph2", bufs=2) as ph2pool,
            tc.tile_pool(name="small", bufs=2) as spool,
            tc.tile_pool(name="uitps", bufs=2, space="PSUM") as uitpool,
            tc.tile_pool(name="xtps", bufs=4, space="PSUM") as xtpspool,
            tc.tile_pool(name="ops", bufs=1, space="PSUM") as opool,
            tc.tile_pool(name="denps", bufs=1, space="PSUM") as denpool,
        ):
            # ---- constants (one-time) ----
            w_f32 = cpool.tile([128, 2 * A], F32, name="w_f32")
            nc.sync.dma_start(out=w_f32[:], in_=w_dram[:])
            w_bf = cpool.tile([128, 2 * A], BF16, name="w_bf")
            nc.vector.tensor_copy(w_bf[:], w_f32[:])

            ident = cpool.tile([128, 128], F32, name="ident")
            nc.sync.dma_start(out=ident[:], in_=ident_dram[:])
            ident_bf = cpool.tile([128, 128], BF16, name="ident_bf")
            nc.vector.tensor_copy(ident_bf[:], ident[:])

            bbt = cpool.tile([1, NCH * A], F32, name="bbt")
            nc.sync.dma_start(out=bbt[:], in_=bbt_dram[:])
            bbt_bf = cpool.tile([1, NCH * A], BF16, name="bbt_bf")
            nc.vector.tensor_copy(bbt_bf[:], bbt[:])
            ones_row = cpool.tile([1, 128], BF16, name="ones_row")
            nc.vector.memset(ones_row[:], 1.0)
            ubt = cpool.tile([128, NCH * A], F32, name="ubt")
            nc.sync.dma_start(out=ubt[:], in_=ubt_dram[:])

            ones_f = cpool.tile([128, 1], F32, name="ones_f")
            nc.vector.memset(ones_f[:], 1.0)

            def emit_chunks(bb):
                uit_ps_halves = []
                x_bf_tiles = []
                for g in range(NG):
                    uit_ps = uitpool.tile([128, RPG * A], F32, name="uit_ps", tag="uit")
                    uit_ps_halves.append(uit_ps)
                    # bias preload: uit_ps = ones^T @ b_row (sets has_written,
                    # so the x@W matmuls accumulate straight onto the bias)
                    nc.tensor.matmul(
                        uit_ps[:],
                        lhsT=ones_row[:],
                        rhs=bbt_bf[:, ds(RPG * A * g, RPG * A)],
                        start=True,
                        stop=False,
                    )
                    x_grp = xfpool.tile([128, RPG, D], F32, name="x_grp", tag="xf")
                    nc.sync.dma_start(
                        out=x_grp[:],
                        in_=x_dram[bb][ds(2048 * g, 2048), :].rearrange(
                            "(p r) d -> p r d", r=RPG
                        ),
                    )
                    x_bf = xbpool.tile([128, RPG, D], BF16, name="x_bf", tag="xb")
                    nc.vector.tensor_copy(x_bf[:], x_grp[:])
                    x_bf_tiles.append(x_bf)

                    # Transpose 4 chunks per PSUM tile, ONE transpose per
                    # chunk: adjacent bf16 pairs are bitcast to f32 elements,
                    # so a [128t x 128pair] f32 transpose relayouts the whole
                    # [128t x 256d] chunk. PSUM tile = 1 bank; one copy per
                    # tile back to SBUF (as bf16, 2x DVE mode), alternating
                    # DVE/ACT.
                    for rp in range(RPG // 4):
                        xt_ps = xtpspool.tile([128, 4, 128], F32, name="xt_ps", tag="xtps")
                        for rr in range(4):
                            r = 4 * rp + rr
                            nc.tensor.transpose(
                                xt_ps[:, rr, :],
                                x_bf[:, r, :].bitcast(F32),
                                ident[:],
                            )
                        xt_sb = xtpool.tile([128, 4, D], BF16, name="xt_sb", tag="xt")
                        if rp % 2 == 0:
                            nc.vector.tensor_copy(xt_sb[:], xt_ps[:].bitcast(BF16))
                        else:
                            nc.scalar.copy(xt_sb[:], xt_ps[:].bitcast(BF16))
                        for rr in range(4):
                            i = 4 * rp + rr
                            xt = xt_sb[:, rr, :].rearrange("p (t s) -> p s t", s=2)
                            nc.tensor.matmul(
                                uit_ps[:, ds(A * i, A)],
                                lhsT=xt[:, 0, :],
                                rhs=w_bf[:, 0:A],
                                start=False,
                                stop=False,
                            )
                            nc.tensor.matmul(
                                uit_ps[:, ds(A * i, A)],
                                lhsT=xt[:, 1, :],
                                rhs=w_bf[:, A : 2 * A],
                                start=False,
                                stop=(i == RPG - 1),
                            )
                return uit_ps_halves, x_bf_tiles

            def emit_tail(bb, uit_ps_halves, x_bf_tiles):
                # ---- phase 2: scores, one half-batch at a time ----
                s_all = spool.tile([128, NCH, 1], F32, name="s_all", tag="s_all")
                for g in range(NG):
                    t2 = ph2pool.tile([128, RPG * A], F32, name="t2", tag="t2")
                    nc.scalar.activation(
                        t2[:], uit_ps_halves[g][:], mybir.ActivationFunctionType.Tanh
                    )
                    t3 = ph2pool.tile([128, RPG * A], F32, name="t3", tag="t3")
                    nc.vector.tensor_mul(t3[:], t2[:], ubt[:, ds(RPG * A * g, RPG * A)])
                    nc.vector.reduce_sum(
                        s_all[:, ds(RPG * g, RPG), :],
                        t3.rearrange("p (i a) -> p i a", a=A),
                        axis=mybir.AxisListType.X,
                    )

                maskb = spool.tile([128, NCH], F32, name="maskb", tag="maskb")
                nc.sync.dma_start(out=maskb[:], in_=maskb_dram[bb])
                s_m = spool.tile([128, NCH], F32, name="s_m", tag="s_m")
                nc.vector.tensor_add(s_m[:], s_all[:, :, 0], maskb[:])

                e_bf = spool.tile([128, NCH], BF16, name="e_bf", tag="e_bf")
                er = spool.tile([128, 1], F32, name="er", tag="er")
                nc.scalar.activation(
                    e_bf[:],
                    s_m[:],
                    mybir.ActivationFunctionType.Exp,
                    accum_out=er[:],
                )

                den_ps = denpool.tile([1, 1], F32, name="den_ps", tag="den")
                nc.tensor.matmul(
                    den_ps[:], lhsT=er[:], rhs=ones_f[:], start=True, stop=True
                )

                # ---- phase 3: weighted sum, two chunks per matmul ----
                # out[2, 512] = [e_i0 e_i1]^T @ [x_i0 | x_i1]; the diagonal
                # blocks (row 0 left, row 1 right) are the real sums, the
                # off-diagonal blocks are discarded.
                o_ps = opool.tile([2, 2 * D], F32, name="o_ps", tag="o")
                for q in range(NCH // 2):
                    g, r0 = divmod(2 * q, RPG)
                    nc.tensor.matmul(
                        o_ps[:],
                        lhsT=e_bf[:, ds(2 * q, 2)],
                        rhs=x_bf_tiles[g][:, r0 : r0 + 2, :],
                        start=(q == 0),
                        stop=(q == NCH // 2 - 1),
                    )

                # ---- phase 4: finalize ----
                den_sb = spool.tile([1, 1], F32, name="den_sb", tag="den_sb")
                nc.vector.tensor_scalar_add(den_sb[:], den_ps[:], EPS)
                inv = spool.tile([1, 1], F32, name="inv", tag="inv")
                nc.vector.reciprocal(inv[:], den_sb[:])
                o2_sb = spool.tile([2, 2 * D], F32, name="o2_sb", tag="o2_sb")
                nc.scalar.copy(o2_sb[:], o_ps[:])
                o_hi = spool.tile([1, D], F32, name="o_hi", tag="o_hi")
                nc.sync.dma_start(out=o_hi[:], in_=o2_sb[1:2, ds(D, D)])
                o_sum = spool.tile([1, D], F32, name="o_sum", tag="o_sum")
                nc.vector.tensor_add(o_sum[:], o2_sb[0:1, 0:D], o_hi[:])
                o_sb = spool.tile([1, D], F32, name="o_sb", tag="o_sb")
                nc.vector.tensor_scalar_mul(o_sb[:], o_sum[:], inv[:])
                nc.sync.dma_start(out=out_dram[bb][None, :], in_=o_sb[:])

            # software-pipeline the emission: batch bb's scores/weighted-sum
            # phases are emitted after batch bb+1's chunk work so the
            # scheduler keeps the PE fed across batch boundaries
            pending = None
            for bb in range(BPC):
                parts = emit_chunks(bb)
                if pending is not None:
                    emit_tail(bb - 1, *pending)
                pending = parts
            emit_tail(BPC - 1, *pending)

    nc.finalize()
    return nc


def kernel(x, mask, W, b, u):
    global last_exec_time_ns
    x = np.ascontiguousarray(np.asarray(x), dtype=np.float32)
    mask_f = np.asarray(mask).astype(np.float32)
    W = np.asarray(W, dtype=np.float32)
    b = np.asarray(b, dtype=np.float32)
    u = np.asarray(u, dtype=np.float32)

    # host-side layout prep (all tiny except x, which is only view-sliced)
    # pair packing: w_packed[j, s*A + a] = W[2j + s, a]
    w_packed = np.ascontiguousarray(W.reshape(128, 2 * A))
    bbt = np.ascontiguousarray(np.tile(b[None, :], (128, NCH)))
    ubt = np.ascontiguousarray(np.tile(u[:, 0][None, :], (128, NCH)))
    # mask -> additive pre-exp bias, laid out [b][p][(g r)] with t = 2048g+16p+r
    maskb = np.ascontiguousarray(
        ((mask_f - 1.0) * MASK_BIAS)
        .reshape(B, NG, 128, RPG)
        .transpose(0, 2, 1, 3)
        .reshape(B, 128, NCH)
    )
    ident = np.eye(128, dtype=np.float32)

    nc = _build()

    in_maps = []
    for c in range(N_CORES):
        in_maps.append(
            {
                "x": x[c * BPC : (c + 1) * BPC],
                "w": w_packed,
                "bbt": bbt,
                "ubt": ubt,
                "maskb": maskb[c * BPC : (c + 1) * BPC],
                "ident": ident,
            }
        )

    trace = bool(int(os.environ.get("BASS_KERNEL_TRACE", "0")))
    res = run_bass_kernel_spmd(
        nc, in_maps, core_ids=list(range(N_CORES)), trace=trace
    )
    last_exec_time_ns = res.exec_time_ns

    out = np.empty((B, D), dtype=np.float32)
    for c in range(N_CORES):
        out[c * BPC : (c + 1) * BPC] = res.results[c]["out"]
    return out
